# revision 5
# baseline (speedup 1.0000x reference)
"""HNet Trainium2 Bass kernel.

Strategy (8 NeuronCores, SPMD via run_bass_kernel_spmd):
  - Each of the 3 transformer blocks (encoder/main/decoder) is split into two
    SPMD launches, token-sharded 8 ways:
      stage1: rmsnorm -> q/kv projections -> rope(q) -> per-position 4x4
              head-mixing attention -> attnout^T (feature-major, per head)
      stage2: head-scramble (relayout) -> wo + residual -> rmsnorm -> top-2
              gate -> 4 experts + shared expert (dense, weighted) + residual
  - Key math facts used:
      * The torch-faithful transpose/reshape "scramble" is a pure relayout:
        storing attention output [H, S, HD]-contiguous and reading it as
        [S, D] reproduces it exactly. Host just re-slices (no flops).
      * The dechunk EMA scan is degenerate: out[b,s] = comp[b, cumsum(mask)-1]
        (a gather), because the chunk vector is constant within a segment.
      * Top-2 gating == gw * (gw >= second_max) / (max + second_max).
  - Matmuls run in float32r (TF32-like, full PE rate, ~1.5e-4 rms error).
  - Router / chunk / dechunk gathers and the scramble re-slicing are cheap
    host-side numpy between launches (data-dependent ragged width L).
"""
import math
import time
import numpy as np

B, S, D = 4, 1024, 512
H, HD = 4, 128
DFF = 4 * D
NEXP = 4
EPS = 1e-6
NCORES = 8

_COMPILED = {}


def _dt():
    import concourse.mybir as mybir
    return mybir


def _mk_nc():
    from concourse import bacc
    return bacc.Bacc("TRN2", target_bir_lowering=False, debug=False,
                     num_devices=NCORES)


# ---------------------------------------------------------------- stage 1
def _build_stage1(T):
    import concourse.tile as tile
    mybir = _dt()
    f32, f32r = mybir.dt.float32, mybir.dt.float32r
    nc = _mk_nc()
    xt = nc.dram_tensor("xt", [D, T], f32, kind="ExternalInput").ap()
    wqT = nc.dram_tensor("wqT", [D, D], f32r, kind="ExternalInput").ap()
    wkvT = nc.dram_tensor("wkvT", [D, 2 * D], f32r, kind="ExternalInput").ap()
    ctab = nc.dram_tensor("ctab", [HD, T], f32, kind="ExternalInput").ap()
    stab = nc.dram_tensor("stab", [HD, T], f32, kind="ExternalInput").ap()
    psig = nc.dram_tensor("psig", [HD, HD], f32, kind="ExternalInput").ap()
    sel = nc.dram_tensor("sel", [128, 16 * 16], f32, kind="ExternalInput").ap()
    gmat = nc.dram_tensor("gmat", [16, 16], f32, kind="ExternalInput").ap()
    ones = nc.dram_tensor("ones", [128, 1], f32, kind="ExternalInput").ap()
    aoT = nc.dram_tensor("aoT", [HD, 4 * T], f32, kind="ExternalOutput").ap()

    with tile.TileContext(nc) as tc:
        with tc.tile_pool(name="wp", bufs=1) as wp, \
             tc.tile_pool(name="sb", bufs=2) as sb, \
             tc.tile_pool(name="pmm", bufs=3, space="PSUM") as pmm, \
             tc.tile_pool(name="pss", bufs=1, space="PSUM") as pss, \
             tc.tile_pool(name="dr", bufs=1, space="DRAM") as dr:
            wq_sb = [[wp.tile([128, 128], f32r, name=f"wq{c}_{m}")
                      for m in range(4)] for c in range(4)]
            wkv_sb = [[wp.tile([128, 128], f32r, name=f"wkv{c}_{m}")
                       for m in range(8)] for c in range(4)]
            for c in range(4):
                for m in range(4):
                    nc.sync.dma_start(out=wq_sb[c][m],
                                      in_=wqT[c * 128:(c + 1) * 128,
                                              m * 128:(m + 1) * 128])
                for m in range(8):
                    nc.sync.dma_start(out=wkv_sb[c][m],
                                      in_=wkvT[c * 128:(c + 1) * 128,
                                               m * 128:(m + 1) * 128])
            ct_sb = wp.tile([HD, T], f32)
            st_sb = wp.tile([HD, T], f32)
            pg_sb = wp.tile([HD, HD], f32)
            sel_sb = wp.tile([128, 16, 16], f32)
            g_sb = wp.tile([16, 16], f32)
            on_sb = wp.tile([128, 1], f32)
            nc.sync.dma_start(out=ct_sb, in_=ctab)
            nc.sync.dma_start(out=st_sb, in_=stab)
            nc.sync.dma_start(out=pg_sb, in_=psig)
            nc.sync.dma_start(out=sel_sb, in_=sel.rearrange("p (r c) -> p r c", r=16))
            nc.sync.dma_start(out=g_sb, in_=gmat)
            nc.sync.dma_start(out=on_sb, in_=ones)

            x_c = [sb.tile([128, T], f32, name=f"x{c}", bufs=1) for c in range(4)]
            for c in range(4):
                nc.sync.dma_start(out=x_c[c], in_=xt[c * 128:(c + 1) * 128, :])

            # rmsnorm
            ssp = pss.tile([1, T], f32, name="ssp")
            for c in range(4):
                sq = sb.tile([128, T], f32, name="sq", tag="sq")
                nc.vector.tensor_mul(sq, x_c[c], x_c[c])
                nc.tensor.matmul(ssp, on_sb, sq, start=(c == 0), stop=(c == 3))
            epsb = sb.tile([1, 1], f32, name="epsb", bufs=1)
            nc.vector.memset(epsb, EPS)
            sd = sb.tile([1, T], f32, name="sd", bufs=1)
            nc.scalar.activation(sd, ssp, mybir.ActivationFunctionType.Sqrt,
                                 bias=epsb, scale=1.0 / D)
            rv = sb.tile([1, T], f32, name="rv", bufs=1)
            nc.vector.reciprocal(rv, sd)
            rb = sb.tile([128, T], f32, name="rb", bufs=1)
            nc.gpsimd.partition_broadcast(rb, rv)
            xn_c = [sb.tile([128, T], f32r, name=f"xn{c}", bufs=1) for c in range(4)]
            for c in range(4):
                nc.vector.tensor_mul(xn_c[c], x_c[c], rb)

            # q / k / v projections (feature-major)
            q_sb = [sb.tile([128, T], f32, name=f"q{m}", bufs=1) for m in range(4)]
            k_sb = [sb.tile([128, T], f32, name=f"k{m}", bufs=1) for m in range(4)]
            v_sb = [sb.tile([128, T], f32, name=f"v{m}", bufs=1) for m in range(4)]
            for m in range(4):
                qp = pmm.tile([128, T], f32, name="qp", tag="mm")
                for c in range(4):
                    nc.tensor.matmul(qp, wq_sb[c][m],
                                     xn_c[c], start=(c == 0), stop=(c == 3))
                nc.scalar.copy(q_sb[m], qp)
            for m in range(8):
                kvp = pmm.tile([128, T], f32, name="kvp", tag="mm")
                for c in range(4):
                    nc.tensor.matmul(kvp, wkv_sb[c][m],
                                     xn_c[c], start=(c == 0), stop=(c == 3))
                nc.scalar.copy(k_sb[m] if m < 4 else v_sb[m - 4], kvp)

            # rope on q (q' = q*ctab + (Psig q)*stab)
            qr_sb = []
            for i in range(4):
                qsp = pmm.tile([128, T], f32, name="qsp", tag="mm")
                nc.tensor.matmul(qsp, pg_sb, q_sb[i], start=True, stop=True)
                tmp = sb.tile([128, T], f32, name="rtmp", tag="rtmp")
                nc.vector.tensor_mul(tmp, st_sb, qsp)
                u = sb.tile([128, T], f32, name="ru", tag="ru")
                nc.vector.tensor_mul(u, q_sb[i], ct_sb)
                qr = sb.tile([128, T], f32, name=f"qr{i}", bufs=1)
                nc.vector.tensor_add(qr, u, tmp)
                qr_sb.append(qr)

            # scores: s16[(i,j), t] = sum_hd qr_i * k_j
            s16 = pss.tile([16, T], f32, name="s16")
            r = 0
            for i in range(4):
                for j in range(4):
                    mp = sb.tile([128, T], f32, name="mp", tag="mp", bufs=3)
                    nc.vector.tensor_mul(mp, qr_sb[i], k_sb[j])
                    nc.tensor.matmul(s16, sel_sb[:, r, :], mp,
                                     start=(r == 0), stop=(r == 15))
                    r += 1
            e16 = sb.tile([16, T], f32, name="e16", bufs=1)
            nc.scalar.activation(e16, s16, mybir.ActivationFunctionType.Exp,
                                 scale=1.0 / math.sqrt(HD))
            d16 = pss.tile([16, T], f32, name="d16")
            nc.tensor.matmul(d16, g_sb, e16, start=True, stop=True)
            dr16 = sb.tile([16, T], f32, name="dr16", bufs=1)
            nc.vector.reciprocal(dr16, d16)
            eh16 = sb.tile([16, T], f32, name="eh16", bufs=1)
            nc.vector.tensor_mul(eh16, e16, dr16)
            eD = dr.tile([16, T], f32, name="eD")
            nc.sync.dma_start(out=eD, in_=eh16)

            # combine: ao_i = sum_j ehat_ij * v_j  (ehat broadcast via DRAM)
            for i in range(4):
                acc = None
                accs = []
                for j in range(4):
                    eb = sb.tile([128, T], f32, name="eb", tag="eb", bufs=4)
                    nc.sync.dma_start(out=eb,
                                      in_=eD[4 * i + j:4 * i + j + 1, :]
                                      .to_broadcast([128, T]))
                    pr = sb.tile([128, T], f32, name="pr", tag="pr", bufs=3)
                    nc.vector.tensor_mul(pr, eb, v_sb[j])
                    accs.append(pr)
                s01 = sb.tile([128, T], f32, name="s01", tag="s01")
                nc.vector.tensor_add(s01, accs[0], accs[1])
                s23 = sb.tile([128, T], f32, name="s23", tag="s23")
                nc.vector.tensor_add(s23, accs[2], accs[3])
                ao = sb.tile([128, T], f32, name="ao", tag="ao", bufs=2)
                nc.vector.tensor_add(ao, s01, s23)
                nc.sync.dma_start(out=aoT[:, i * T:(i + 1) * T], in_=ao)
    nc.compile()
    return nc


# ---------------------------------------------------------------- stage 2
def _build_stage2(T2):
    import concourse.tile as tile
    mybir = _dt()
    f32, f32r = mybir.dt.float32, mybir.dt.float32r
    nc = _mk_nc()
    scrT = nc.dram_tensor("scrT", [D, T2], f32r, kind="ExternalInput").ap()
    xres = nc.dram_tensor("xres", [D, T2], f32, kind="ExternalInput").ap()
    woT = nc.dram_tensor("woT", [D, D], f32r, kind="ExternalInput").ap()
    gateT = nc.dram_tensor("gateT", [D, 4], f32r, kind="ExternalInput").ap()
    w1T = nc.dram_tensor("w1T", [5, D, DFF], f32r, kind="ExternalInput").ap()
    w3T = nc.dram_tensor("w3T", [5, D, DFF], f32r, kind="ExternalInput").ap()
    w2T = nc.dram_tensor("w2T", [5, DFF, D], f32r, kind="ExternalInput").ap()
    ones = nc.dram_tensor("ones", [128, 1], f32, kind="ExternalInput").ap()
    outT = nc.dram_tensor("outT", [D, T2], f32, kind="ExternalOutput").ap()

    with tile.TileContext(nc) as tc:
        with tc.tile_pool(name="wp", bufs=1) as wp, \
             tc.tile_pool(name="sb", bufs=2) as sb, \
             tc.tile_pool(name="ws", bufs=3) as ws, \
             tc.tile_pool(name="dr", bufs=1, space="DRAM") as dr:
            wo_sb = [[wp.tile([128, 128], f32r, name=f"wo{c}_{m}")
                      for m in range(4)] for c in range(4)]
            gt_sb = [wp.tile([128, 4], f32r, name=f"gt{c}") for c in range(4)]
            on_sb = wp.tile([128, 1], f32)
            nc.sync.dma_start(out=on_sb, in_=ones)
            scr_c = [wp.tile([128, T2], f32r, name=f"scr{c}") for c in range(4)]
            xr_c = [wp.tile([128, T2], f32, name=f"xr{c}") for c in range(4)]
            for c in range(4):
                for m in range(4):
                    nc.sync.dma_start(out=wo_sb[c][m],
                                      in_=woT[c * 128:(c + 1) * 128,
                                              m * 128:(m + 1) * 128])
                nc.sync.dma_start(out=gt_sb[c], in_=gateT[c * 128:(c + 1) * 128, :])
                nc.sync.dma_start(out=scr_c[c], in_=scrT[c * 128:(c + 1) * 128, :])
                nc.sync.dma_start(out=xr_c[c], in_=xres[c * 128:(c + 1) * 128, :])

            mla_c = [sb.tile([128, T2], f32, name=f"mla{c}", bufs=1) for c in range(4)]
            with tc.tile_pool(name="ppre", bufs=2, space="PSUM") as ppre:
                for m in range(4):
                    wop = ppre.tile([128, T2], f32, name="wop", tag="wop")
                    for c in range(4):
                        nc.tensor.matmul(wop, wo_sb[c][m],
                                         scr_c[c], start=(c == 0), stop=(c == 3))
                    nc.vector.tensor_add(mla_c[m], xr_c[m], wop)

                # rmsnorm 2
                ssp = ppre.tile([1, T2], f32, name="ssp", tag="ss", bufs=1)
                for c in range(4):
                    sq = sb.tile([128, T2], f32, name="sq", tag="sq")
                    nc.vector.tensor_mul(sq, mla_c[c], mla_c[c])
                    nc.tensor.matmul(ssp, on_sb, sq, start=(c == 0), stop=(c == 3))
                epsb = sb.tile([1, 1], f32, name="epsb", bufs=1)
                nc.vector.memset(epsb, EPS)
                sd = sb.tile([1, T2], f32, name="sd", bufs=1)
                nc.scalar.activation(sd, ssp, mybir.ActivationFunctionType.Sqrt,
                                     bias=epsb, scale=1.0 / D)
                rv = sb.tile([1, T2], f32, name="rv", bufs=1)
                nc.vector.reciprocal(rv, sd)
                rb = sb.tile([128, T2], f32, name="rb", bufs=1)
                nc.gpsimd.partition_broadcast(rb, rv)
                xn_c = [sb.tile([128, T2], f32r, name=f"xn{c}", bufs=1)
                        for c in range(4)]
                for c in range(4):
                    nc.vector.tensor_mul(xn_c[c], mla_c[c], rb)

                # top-2 gate -> per-expert weights we[e] (4 rows)
                gp = ppre.tile([4, T2], f32, name="gp", tag="gp", bufs=1)
                for c in range(4):
                    nc.tensor.matmul(gp, gt_sb[c], xn_c[c],
                                     start=(c == 0), stop=(c == 3))
                gu = sb.tile([4, T2], f32, name="gu", bufs=1)
                nc.scalar.activation(gu, gp, mybir.ActivationFunctionType.Exp)
                import concourse.bass_isa as bass_isa
                m1 = sb.tile([4, T2], f32, name="m1", bufs=1)
                nc.gpsimd.partition_all_reduce(m1, gu, channels=4,
                                               reduce_op=bass_isa.ReduceOp.max)
                eqm = sb.tile([4, T2], f32, name="eqm", bufs=1)
                nc.vector.tensor_tensor(eqm, gu, m1, op=mybir.AluOpType.is_equal)
                msk = sb.tile([4, T2], f32, name="msk", bufs=1)
                nc.vector.scalar_tensor_tensor(msk, eqm, -1e30, gu,
                                               op0=mybir.AluOpType.mult,
                                               op1=mybir.AluOpType.add)
                m2 = sb.tile([4, T2], f32, name="m2", bufs=1)
                nc.gpsimd.partition_all_reduce(m2, msk, channels=4,
                                               reduce_op=bass_isa.ReduceOp.max)
                geq = sb.tile([4, T2], f32, name="geq", bufs=1)
                nc.vector.tensor_tensor(geq, gu, m2, op=mybir.AluOpType.is_ge)
                num = sb.tile([4, T2], f32, name="num", bufs=1)
                nc.vector.tensor_mul(num, geq, gu)
                den = sb.tile([4, T2], f32, name="den", bufs=1)
                nc.vector.tensor_add(den, m1, m2)
                drg = sb.tile([4, T2], f32, name="drg", bufs=1)
                nc.vector.reciprocal(drg, den)
                wev = sb.tile([4, T2], f32, name="wev", bufs=1)
                nc.vector.tensor_mul(wev, num, drg)
                weD = dr.tile([4, T2], f32, name="weD")
                nc.sync.dma_start(out=weD, in_=wev)

            web = []
            for e in range(4):
                wb = sb.tile([128, T2], f32, name=f"web{e}", bufs=1)
                nc.sync.dma_start(out=wb, in_=weD[e:e + 1, :].to_broadcast([128, T2]))
                web.append(wb)

            # experts (4 routed + 1 shared), accumulate into mla_c in-place
            with tc.tile_pool(name="pop", bufs=1, space="PSUM") as pop, \
                 tc.tile_pool(name="ph", bufs=2, space="PSUM") as ph:
                for e in range(5):
                    op_c = [pop.tile([128, T2], f32, name=f"op{c}", tag=f"op{c}")
                            for c in range(4)]
                    for f in range(16):
                        w1s = [ws.tile([128, 128], f32r, name=f"w1s{c}",
                                       tag=f"w1s{c}") for c in range(4)]
                        w3s = [ws.tile([128, 128], f32r, name=f"w3s{c}",
                                       tag=f"w3s{c}") for c in range(4)]
                        w2s = [ws.tile([128, 128], f32r, name=f"w2s{c}",
                                       tag=f"w2s{c}") for c in range(4)]
                        for c in range(4):
                            nc.sync.dma_start(
                                out=w1s[c],
                                in_=w1T[e][c * 128:(c + 1) * 128,
                                           f * 128:(f + 1) * 128])
                            nc.sync.dma_start(
                                out=w3s[c],
                                in_=w3T[e][c * 128:(c + 1) * 128,
                                           f * 128:(f + 1) * 128])
                            nc.sync.dma_start(
                                out=w2s[c],
                                in_=w2T[e][f * 128:(f + 1) * 128,
                                           c * 128:(c + 1) * 128])
                        h1p = ph.tile([128, T2], f32, name="h1p", tag="h1")
                        h3p = ph.tile([128, T2], f32, name="h3p", tag="h3")
                        for c in range(4):
                            nc.tensor.matmul(h1p, w1s[c], xn_c[c],
                                             start=(c == 0), stop=(c == 3))
                        for c in range(4):
                            nc.tensor.matmul(h3p, w3s[c], xn_c[c],
                                             start=(c == 0), stop=(c == 3))
                        g1 = sb.tile([128, T2], f32, name="g1", tag="g1")
                        nc.scalar.activation(g1, h1p,
                                             mybir.ActivationFunctionType.Gelu)
                        hg = sb.tile([128, T2], f32r, name="hg", tag="hg", bufs=3)
                        nc.vector.tensor_mul(hg, g1, h3p)
                        for c in range(4):
                            nc.tensor.matmul(op_c[c], w2s[c],
                                             hg, start=(f == 0), stop=(f == 15))
                    if e < 4:
                        for c in range(4):
                            tmp = sb.tile([128, T2], f32, name="etmp", tag="etmp")
                            nc.vector.tensor_mul(tmp, web[e], op_c[c])
                            nc.vector.tensor_add(mla_c[c], mla_c[c], tmp)
                    else:
                        for c in range(4):
                            nc.vector.tensor_add(mla_c[c], mla_c[c], op_c[c])
            for c in range(4):
                nc.sync.dma_start(out=outT[c * 128:(c + 1) * 128, :], in_=mla_c[c])
    nc.compile()
    return nc


def _get_compiled(kind, T):
    key = (kind, T)
    if key not in _COMPILED:
        _COMPILED[key] = (_build_stage1 if kind == "s1" else _build_stage2)(T)
    return _COMPILED[key]


def _run(nc, in_maps, timings=None):
    from concourse import bass_utils
    last = None
    for attempt in range(3):
        try:
            t0 = time.time()
            res = bass_utils.run_bass_kernel_spmd(
                nc, in_maps, core_ids=list(range(NCORES)), trace=False)
            if timings is not None:
                timings.append(int((time.time() - t0) * 1e9))
            return res
        except Exception as e:  # transient device wedges: retry
            last = e
            time.sleep(3.0)
    raise last


# ------------------------------------------------------------- host glue
def _rope_tabs(positions):
    """positions: int array [T] -> ctab, stab [128, T] fp32."""
    theta = 1.0 / (10000.0 ** (np.arange(0, HD, 2, dtype=np.float32) / HD))
    ang = positions[:, None].astype(np.float32) * theta[None, :]  # [T, 64]
    c = np.cos(ang).T  # [64, T]
    s = np.sin(ang).T
    ctab = np.repeat(c, 2, axis=0).astype(np.float32)
    stab = np.empty((HD, positions.shape[0]), np.float32)
    stab[0::2] = -s
    stab[1::2] = s
    return np.ascontiguousarray(ctab), np.ascontiguousarray(stab)


def _consts():
    sel = np.zeros((128, 16, 16), np.float32)
    for r in range(16):
        sel[:, r, r] = 1.0
    g = np.zeros((16, 16), np.float32)
    for i in range(4):
        for j in range(4):
            for j2 in range(4):
                g[i * 4 + j, i * 4 + j2] = 1.0
    psig = np.zeros((HD, HD), np.float32)
    for m in range(HD // 2):
        psig[2 * m, 2 * m + 1] = 1.0
        psig[2 * m + 1, 2 * m] = 1.0
    ones = np.ones((128, 1), np.float32)
    return (np.ascontiguousarray(sel.reshape(128, 256)), g, psig, ones)


def _prep_block(bp):
    """Host-side weight relayout for one block."""
    w = {}
    wq = np.asarray(bp['mla']['wq'], np.float32)
    wkv = np.asarray(bp['mla']['wkv'], np.float32)
    wo = np.asarray(bp['mla']['wo'], np.float32)
    # reorder wkv rows into [k heads | v heads]
    kv = wkv.reshape(H, 2 * HD, D)
    wkv_re = np.concatenate([kv[:, :HD, :].reshape(H * HD, D),
                             kv[:, HD:, :].reshape(H * HD, D)], axis=0)
    w['wqT'] = np.ascontiguousarray(wq.T)
    w['wkvT'] = np.ascontiguousarray(wkv_re.T)
    w['woT'] = np.ascontiguousarray(wo.T)
    w['gateT'] = np.ascontiguousarray(np.asarray(bp['moe']['gate'], np.float32).T)
    exps = list(bp['moe']['experts']) + [bp['moe']['shared']]
    w['w1T'] = np.ascontiguousarray(np.stack(
        [np.asarray(ep['w1'], np.float32).T for ep in exps]))
    w['w3T'] = np.ascontiguousarray(np.stack(
        [np.asarray(ep['w3'], np.float32).T for ep in exps]))
    w['w2T'] = np.ascontiguousarray(np.stack(
        [np.asarray(ep['w2'], np.float32).T for ep in exps]))
    return w


def _block_device(xfull, wblk, consts, Lrow, timings):
    """Run one block on device. xfull: [B, Lrow, D] fp32. Returns [B, Lrow, D]."""
    sel, g, psig, ones = consts
    nb = xfull.shape[0]
    ntok = nb * Lrow
    T1 = (ntok + NCORES - 1) // NCORES
    T1 = (T1 + 3) // 4 * 4  # f32r matmuls need even moving free dim
    nc1 = _get_compiled("s1", T1)
    # flat tokens, b-major
    xflat = xfull.reshape(ntok, D)
    in_maps = []
    for c in range(NCORES):
        lo = c * T1
        idx = np.arange(lo, lo + T1)
        valid = idx < ntok
        idxc = np.where(valid, idx, 0)
        xt = np.ascontiguousarray(xflat[idxc].T)
        xt[:, ~valid] = 0.0
        pos = np.where(valid, idxc % Lrow, 0)
        ctab, stab = _rope_tabs(pos)
        in_maps.append(dict(xt=xt, wqT=wblk['wqT'], wkvT=wblk['wkvT'],
                            ctab=ctab, stab=stab, psig=psig, sel=sel,
                            gmat=g, ones=ones))
    res = _run(nc1, in_maps, timings)
    # assemble virtual attn buffer per row: virt[b] [128, 4*Lrow]
    virt = np.zeros((nb, HD, 4 * Lrow), np.float32)
    for c in range(NCORES):
        ao = res.results[c]["aoT"]  # [128, 4*T1], head-major blocks of T1
        lo = c * T1
        hi = min(lo + T1, ntok)
        if hi <= lo:
            continue
        n = hi - lo
        for h in range(H):
            blk = ao[:, h * T1:h * T1 + n]  # [128, n] tokens lo..hi
            fts = np.arange(lo, hi)
            bs = fts // Lrow
            ss = fts % Lrow
            for b in np.unique(bs):
                m = bs == b
                virt[b][:, h * Lrow + ss[m]] = blk[:, m]
    # stage 2
    T2 = T1
    nc2 = _get_compiled("s2", T2)
    in_maps2 = []
    vr = np.transpose(virt, (0, 2, 1))  # [nb, 4*Lrow, 128] rows=virtual cols
    for c in range(NCORES):
        lo = c * T2
        idx = np.arange(lo, lo + T2)
        valid = idx < ntok
        idxc = np.where(valid, idx, 0)
        bs = idxc // Lrow
        ss = idxc % Lrow
        rows = vr[bs[:, None], (4 * ss)[:, None] + np.arange(4)[None, :], :]
        # rows: [T2, 4, 128] -> scrT [4*128, T2]
        scr = np.ascontiguousarray(rows.transpose(1, 2, 0).reshape(D, T2))
        scr[:, ~valid] = 0.0
        xr = np.ascontiguousarray(xflat[idxc].T)
        xr[:, ~valid] = 0.0
        in_maps2.append(dict(scrT=scr, xres=xr, woT=wblk['woT'],
                             gateT=wblk['gateT'], w1T=wblk['w1T'],
                             w3T=wblk['w3T'], w2T=wblk['w2T'], ones=ones))
    res2 = _run(nc2, in_maps2, timings)
    out = np.zeros((ntok, D), np.float32)
    for c in range(NCORES):
        lo = c * T2
        hi = min(lo + T2, ntok)
        if hi <= lo:
            continue
        out[lo:hi] = res2.results[c]["outT"][:, :hi - lo].T
    return out.reshape(nb, Lrow, D)


def _router_host(enc, rp):
    wq = np.asarray(rp['wq'], np.float32)
    wk = np.asarray(rp['wk'], np.float32)
    ident = (wq.shape == (D, D) and np.array_equal(wq, np.eye(D, dtype=np.float32)))
    q = enc[:, :-1] if ident else enc[:, :-1] @ wq.T
    identk = (wk.shape == (D, D) and np.array_equal(wk, np.eye(D, dtype=np.float32)))
    k = enc[:, 1:] if identk else enc[:, 1:] @ wk.T
    qn = q / np.maximum(np.linalg.norm(q, axis=-1, keepdims=True), 1e-12)
    kn = k / np.maximum(np.linalg.norm(k, axis=-1, keepdims=True), 1e-12)
    cos = np.sum(qn * kn, -1)
    bp = np.clip((1.0 - cos) / 2.0, 0.0, 1.0)
    bp = np.pad(bp, ((0, 0), (1, 0)), constant_values=1.0)
    mask = bp > 0.5  # argmax([1-bp, bp]) == 1
    mask[:, 0] = True
    return mask


def kernel(x, params):
    x = np.asarray(x, np.float32)
    consts = _consts()
    wenc = _prep_block(params['encoder'])
    wmain = _prep_block(params['main'])
    wdec = _prep_block(params['decoder'])
    timings = []

    enc = _block_device(x, wenc, consts, S, timings)
    mask = _router_host(enc, params['router'])
    nbound = mask.sum(1)
    L = int(nbound.max())
    comp = np.zeros((B, L, D), np.float32)
    for b in range(B):
        idx = np.nonzero(mask[b])[0]
        comp[b, :len(idx)] = enc[b, idx]
    main = _block_device(comp, wmain, consts, L, timings)
    cum = np.cumsum(mask.astype(np.int32), axis=1)
    dec_in = np.take_along_axis(main, (cum - 1)[..., None], axis=1)
    out = _block_device(dec_in, wdec, consts, S, timings)
    if timings is not None:
        kernel.last_hw_ns = sum(timings)
        kernel.launch_ns = list(timings)
    return out.astype(np.float32)
